# revision 28
# baseline (speedup 1.0000x reference)
"""Trainium2 Bass kernel for nn_DifferentiableKnnGraphLayer.

Reference computation (per batch b, per node-row i — rows fully independent):
  scores = 5*tanh(logits/5);  g0 = scores + gumbel[b]
  repeat K=20: g += log(clip(1-onehot, 1e-20)); onehot = softmax(g/0.5); khot += onehot
  out_weight[b, i*N+j] = ((1+khot)-khot) at top-20(khot) positions else 0
  edge_index = [tile(arange(N), N), repeat(arange(N), N)]  (input-independent)

Kernel strategy:
  * Shard the 16384 independent rows across 8 cores: core c owns logits rows
    [c*256,(c+1)*256) for all 8 batches -> 16 tiles of [128, 2048] per core.
  * Multiplicative softmax-iteration: e <- e*(1-onehot)^2 is exact-math equal
    to the reference masking (softmax is shift/scale invariant; fp32 range is
    sufficient without renormalization).
  * Engine balance per iteration: DVE computes onehot=e*r (2x-mode
    tensor_scalar) and the fused e=(e*1)*w2 with accumulated row-sum
    (scalar_tensor_tensor+accum); ACT computes w2=Square(1-r*e) directly from
    e via per-partition scale=-r (runs concurrently with the DVE passes);
    GPSIMD accumulates khot.
  * Top-20 selection by per-row threshold bisection (22 probes resolve
    ~1.2e-7 intervals vs measured min top-20/21 margins ~4e-7), probes on ACT:
    accum-sum of Sign(mid - khot) gives the >mid count; the bisection handles
    elements exactly equal to a probe threshold correctly.
  * Output pass: (khot > t) * ((1+khot) - khot), fused scalar_tensor_tensor,
    reproducing the reference's fp32 rounding exactly (0 off-selection).
"""
import sys

sys.path.insert(0, "/opt/trn_rl_repo")

import numpy as np

N = 2048
B = 8
K = 20
NCORES = 8
RPC = N // NCORES        # logits rows per core = 256
NBLK = RPC // 128        # 128-row blocks per core = 2
NT = NBLK * B            # tiles per core = 16
NPROBE = 22
G = 1                    # tiles per bisection group

_cached = {}


def _build_program(nblk=NBLK, nb=B, ncores=NCORES, debug_out=False):
    import concourse.bass as bass
    import concourse.bacc as bacc
    import concourse.tile as tile
    from concourse import mybir

    f32 = mybir.dt.float32
    Alu = mybir.AluOpType
    Act = mybir.ActivationFunctionType

    nt = nblk * nb
    assert nt % G == 0 or nt < G
    g_size = min(G, nt)
    nc = bacc.Bacc("TRN2", target_bir_lowering=False, debug=False, num_devices=ncores)
    logits_s = nc.dram_tensor("logits_s", [nblk, 128, N], f32, kind="ExternalInput").ap()
    gumbel_s = nc.dram_tensor("gumbel_s", [nt, 128, N], f32, kind="ExternalInput").ap()
    out_s = nc.dram_tensor("out_s", [nt, 128, N], f32, kind="ExternalOutput").ap()

    with tile.TileContext(nc) as tc:
        with (
            tc.tile_pool(name="scores", bufs=2) as scores_pool,
            tc.tile_pool(name="work", bufs=5) as work_pool,
            tc.tile_pool(name="oh", bufs=3) as oh_pool,
            tc.tile_pool(name="w2", bufs=3) as w2_pool,
            tc.tile_pool(name="sgn", bufs=2) as sgn_pool,
            tc.tile_pool(name="opool", bufs=1) as o_pool,
            tc.tile_pool(name="khot", bufs=7) as khot_pool,
            tc.tile_pool(name="small", bufs=12) as small_pool,
        ):
            s5_tiles = {}
            neg1 = small_pool.tile([128, 1], f32, tag="neg1")
            nc.vector.memset(neg1[:], -1.0)

            def main_loop(t_idx):
                blk = t_idx // nb
                if t_idx % nb == 0:
                    lt = work_pool.tile([128, N], f32, tag="g")
                    nc.sync.dma_start(lt[:], logits_s[blk])
                    s5 = scores_pool.tile([128, N], f32)
                    # scores5 = 5 * tanh(logits * 0.2)
                    nc.scalar.activation(s5[:], lt[:], Act.Tanh, scale=0.2)
                    nc.scalar.mul(s5[:], s5[:], 5.0)
                    s5_tiles[blk] = s5
                s5 = s5_tiles[blk]
                g = work_pool.tile([128, N], f32, tag="g")
                nc.sync.dma_start(g[:], gumbel_s[t_idx])
                m = small_pool.tile([128, 1], f32, tag="m")
                # g0 = gumbel + scores5 ; m = rowmax(g0)
                nc.vector.tensor_tensor(out=g[:], in0=g[:], in1=s5[:], op=Alu.add)
                nc.vector.tensor_reduce(
                    out=m[:], in_=g[:], axis=mybir.AxisListType.X, op=Alu.max
                )
                bias = small_pool.tile([128, 1], f32, tag="bias")
                nc.vector.tensor_scalar_mul(bias[:], m[:], -2.0)
                # e = exp(2*g0 - 2*m) ; s = rowsum(e)   (in-place on g)
                s = small_pool.tile([128, 1], f32, tag="s")
                nc.scalar.activation(
                    g[:], g[:], Act.Exp, bias=bias[:], scale=2.0, accum_out=s[:]
                )
                e = g
                khot = khot_pool.tile([128, N], f32)
                for k in range(K):
                    r = small_pool.tile([128, 1], f32, tag="r")
                    nc.vector.reciprocal(r[:], s[:])
                    if k == 0:
                        oh = khot  # write the first onehot straight into khot
                    else:
                        oh = oh_pool.tile([128, N], f32, tag="oh")
                    # onehot = e * r  (tensor_scalar -> fp32 2x mode)
                    nc.vector.tensor_scalar(
                        out=oh[:], in0=e[:], scalar1=r[:], scalar2=None, op0=Alu.mult
                    )
                    if k > 0:
                        nc.gpsimd.tensor_add(khot[:], khot[:], oh[:])
                    if k < K - 1:
                        w2 = w2_pool.tile([128, N], f32, tag="w2")
                        # w2 = (e*r - 1)^2 == (1 - e*r)^2: reads e (not onehot)
                        # so ACT runs concurrently with the DVE onehot pass
                        nc.scalar.activation(
                            w2[:], e[:], Act.Square, bias=neg1[:], scale=r[:]
                        )
                        s = small_pool.tile([128, 1], f32, tag="s")
                        # e = (e * 1) * w2 (in place) ; s = rowsum(e)
                        nc.vector.scalar_tensor_tensor(
                            out=e[:], in0=e[:], scalar=1.0, in1=w2[:],
                            op0=Alu.mult, op1=Alu.mult, accum_out=s[:],
                        )
                return khot

            def bisect_one(t_idx, khot):
                lo = None
                for p in range(NPROBE):
                    c_p = 0.5 * (2.0 ** -(p + 1))
                    mid = small_pool.tile([128, 1], f32, tag="mid")
                    if lo is None:
                        nc.vector.memset(mid[:], c_p)
                    else:
                        nc.vector.tensor_scalar(
                            out=mid[:], in0=lo[:], scalar1=c_p, scalar2=None,
                            op0=Alu.add,
                        )
                    sgn = sgn_pool.tile([128, N], f32, tag="sgn")
                    cnt = small_pool.tile([128, 1], f32, tag="cnt")
                    # S' = sum(sign(mid - khot)); count(>mid)>=20 <=> S'<=N-2K
                    nc.scalar.activation(
                        sgn[:], khot[:], Act.Sign, bias=mid[:],
                        scale=-1.0, accum_out=cnt[:],
                    )
                    ge = small_pool.tile([128, 1], f32, tag="ge")
                    nc.vector.tensor_scalar(
                        out=ge[:], in0=cnt[:], scalar1=float(N - 2 * K),
                        scalar2=None, op0=Alu.is_le,
                    )
                    lo2 = small_pool.tile([128, 1], f32, tag="lo")
                    if lo is None:
                        nc.vector.tensor_scalar(
                            out=lo2[:], in0=ge[:], scalar1=c_p, scalar2=None,
                            op0=Alu.mult,
                        )
                    else:
                        nc.vector.scalar_tensor_tensor(
                            out=lo2[:], in0=ge[:], scalar=c_p, in1=lo[:],
                            op0=Alu.mult, op1=Alu.add,
                        )
                    lo = lo2
                v = oh_pool.tile([128, N], f32, tag="oh")
                # v = (khot + 1) - khot
                nc.vector.scalar_tensor_tensor(
                    out=v[:], in0=khot[:], scalar=1.0, in1=khot[:],
                    op0=Alu.add, op1=Alu.subtract,
                )
                o = o_pool.tile([128, N], f32, tag="o")
                # o = (khot > lo) * v
                nc.vector.scalar_tensor_tensor(
                    out=o[:], in0=khot[:], scalar=lo[:], in1=v[:],
                    op0=Alu.is_gt, op1=Alu.mult,
                )
                nc.sync.dma_start(out_s[t_idx], o[:])

            # stagger: emit tile t's bisection only after tile t+2's main
            # loop so the static per-engine order interleaves each bisect
            # chain with the following tiles' compute
            from collections import deque
            pending = deque()
            for t_idx in range(nt):
                khot = main_loop(t_idx)
                pending.append((t_idx, khot))
                if len(pending) > 2:
                    bisect_one(*pending.popleft())
            while pending:
                bisect_one(*pending.popleft())

    nc.compile()
    return nc


def _get_program():
    if "nc" not in _cached:
        _cached["nc"] = _build_program()
    return _cached["nc"]


def kernel(logits, gumbel, x=None, emb=None, _trace=False):
    from concourse.bass_utils import run_bass_kernel_spmd

    logits = np.ascontiguousarray(logits, dtype=np.float32)
    gumbel = np.ascontiguousarray(gumbel, dtype=np.float32)

    nc = _get_program()
    in_maps = []
    for c in range(NCORES):
        r0 = c * RPC
        lg = logits[r0:r0 + RPC].reshape(NBLK, 128, N)
        # tile t = blk*B + b  ->  gumbel[b, r0+blk*128 : r0+(blk+1)*128, :]
        gm = np.ascontiguousarray(
            gumbel[:, r0:r0 + RPC, :]            # [B, 256, N]
            .reshape(B, NBLK, 128, N)
            .transpose(1, 0, 2, 3)               # [NBLK, B, 128, N]
            .reshape(NT, 128, N)
        )
        in_maps.append({"logits_s": np.ascontiguousarray(lg), "gumbel_s": gm})

    res = None
    last_err = None
    for attempt in range(3):
        try:
            res = run_bass_kernel_spmd(nc, in_maps, list(range(NCORES)), trace=_trace)
            break
        except Exception as err:  # transient NRT/device failures: retry
            last_err = err
            import time as _time
            _time.sleep(5 * (attempt + 1))
    if res is None:
        raise last_err

    edge_weight = np.empty((B, N, N), dtype=np.float32)
    for c in range(NCORES):
        r0 = c * RPC
        o = res.results[c]["out_s"].reshape(NBLK, B, 128, N)
        edge_weight[:, r0:r0 + RPC, :] = o.transpose(1, 0, 2, 3).reshape(B, RPC, N)

    idx = np.arange(N, dtype=np.int32)
    edge_index = np.stack([np.tile(idx, N), np.repeat(idx, N)])
    out = (edge_index, edge_weight.reshape(B, N * N))
    if _trace:
        return out, res
    return out


# revision 30
# speedup vs baseline: 1.0145x; 1.0145x over previous
"""Trainium2 Bass kernel for nn_DifferentiableKnnGraphLayer.

Reference computation (per batch b, per node-row i — rows fully independent):
  scores = 5*tanh(logits/5);  g0 = scores + gumbel[b]
  repeat K=20: g += log(clip(1-onehot, 1e-20)); onehot = softmax(g/0.5); khot += onehot
  out_weight[b, i*N+j] = ((1+khot)-khot) at top-20(khot) positions else 0
  edge_index = [tile(arange(N), N), repeat(arange(N), N)]  (input-independent)

Kernel strategy:
  * Shard the 16384 independent rows across 8 cores: core c owns logits rows
    [c*256,(c+1)*256) for all 8 batches -> 16 tiles of [128, 2048] per core.
  * Multiplicative softmax-iteration: e <- e*(1-onehot)^2 is exact-math equal
    to the reference masking (softmax is shift/scale invariant; fp32 range is
    sufficient without renormalization).
  * Engine balance per iteration: DVE computes onehot=e*r (2x-mode
    tensor_scalar) and the fused e=(e*1)*w2 with accumulated row-sum
    (scalar_tensor_tensor+accum); ACT computes w2=Square(1-r*e) directly from
    e via per-partition scale=-r (runs concurrently with the DVE passes);
    GPSIMD accumulates khot.
  * Top-20 selection by per-row threshold bisection (22 probes resolve
    ~1.2e-7 intervals vs measured min top-20/21 margins ~4e-7), probes on ACT:
    accum-sum of Sign(mid - khot) gives the >mid count; the bisection handles
    elements exactly equal to a probe threshold correctly.
  * Output pass: (khot > t) * ((1+khot) - khot), fused scalar_tensor_tensor,
    reproducing the reference's fp32 rounding exactly (0 off-selection).
"""
import sys

sys.path.insert(0, "/opt/trn_rl_repo")

import numpy as np

N = 2048
B = 8
K = 20
NCORES = 8
RPC = N // NCORES        # logits rows per core = 256
NBLK = RPC // 128        # 128-row blocks per core = 2
NT = NBLK * B            # tiles per core = 16
NPROBE = 22
G = 1                    # tiles per bisection group

_cached = {}


def _build_program(nblk=NBLK, nb=B, ncores=NCORES, debug_out=False):
    import concourse.bass as bass
    import concourse.bacc as bacc
    import concourse.tile as tile
    from concourse import mybir

    f32 = mybir.dt.float32
    Alu = mybir.AluOpType
    Act = mybir.ActivationFunctionType

    nt = nblk * nb
    assert nt % G == 0 or nt < G
    g_size = min(G, nt)
    nc = bacc.Bacc("TRN2", target_bir_lowering=False, debug=False, num_devices=ncores)
    logits_s = nc.dram_tensor("logits_s", [nblk, 128, N], f32, kind="ExternalInput").ap()
    gumbel_s = nc.dram_tensor("gumbel_s", [nt, 128, N], f32, kind="ExternalInput").ap()
    out_s = nc.dram_tensor("out_s", [nt, 128, N], f32, kind="ExternalOutput").ap()

    with tile.TileContext(nc) as tc:
        with (
            tc.tile_pool(name="scores", bufs=2) as scores_pool,
            tc.tile_pool(name="work", bufs=5) as work_pool,
            tc.tile_pool(name="oh", bufs=3) as oh_pool,
            tc.tile_pool(name="w2", bufs=3) as w2_pool,
            tc.tile_pool(name="sgn", bufs=2) as sgn_pool,
            tc.tile_pool(name="opool", bufs=1) as o_pool,
            tc.tile_pool(name="khot", bufs=7) as khot_pool,
            tc.tile_pool(name="small", bufs=12) as small_pool,
        ):
            s5_tiles = {}
            neg1 = small_pool.tile([128, 1], f32, tag="neg1")
            nc.vector.memset(neg1[:], -1.0)

            def main_loop(t_idx):
                blk = t_idx // nb
                if t_idx % nb == 0:
                    lt = work_pool.tile([128, N], f32, tag="g")
                    nc.sync.dma_start(lt[:], logits_s[blk])
                    s5 = scores_pool.tile([128, N], f32)
                    # scores5 = 5 * tanh(logits * 0.2)
                    nc.scalar.activation(s5[:], lt[:], Act.Tanh, scale=0.2)
                    nc.scalar.mul(s5[:], s5[:], 5.0)
                    s5_tiles[blk] = s5
                s5 = s5_tiles[blk]
                g = work_pool.tile([128, N], f32, tag="g")
                nc.sync.dma_start(g[:], gumbel_s[t_idx])
                m = small_pool.tile([128, 1], f32, tag="m")
                # g0 = gumbel + scores5 ; m = rowmax(g0)
                nc.vector.tensor_tensor(out=g[:], in0=g[:], in1=s5[:], op=Alu.add)
                nc.vector.tensor_reduce(
                    out=m[:], in_=g[:], axis=mybir.AxisListType.X, op=Alu.max
                )
                bias = small_pool.tile([128, 1], f32, tag="bias")
                nc.vector.tensor_scalar_mul(bias[:], m[:], -2.0)
                # e = exp(2*g0 - 2*m) ; s = rowsum(e)   (in-place on g)
                s = small_pool.tile([128, 1], f32, tag="s")
                nc.scalar.activation(
                    g[:], g[:], Act.Exp, bias=bias[:], scale=2.0, accum_out=s[:]
                )
                e = g
                khot = khot_pool.tile([128, N], f32)
                for k in range(K):
                    r = small_pool.tile([128, 1], f32, tag="r")
                    nc.vector.reciprocal(r[:], s[:])
                    if k < K - 1:
                        w2 = w2_pool.tile([128, N], f32, tag="w2")
                        # w2 = (e*r - 1)^2 == (1 - e*r)^2: reads e (not onehot);
                        # emitted FIRST so ACT starts it before servicing
                        # bisection probes -- the DVE e-update waits on it
                        nc.scalar.activation(
                            w2[:], e[:], Act.Square, bias=neg1[:], scale=r[:]
                        )
                    if k == 0:
                        oh = khot  # write the first onehot straight into khot
                    else:
                        oh = oh_pool.tile([128, N], f32, tag="oh")
                    # onehot = e * r  (tensor_scalar -> fp32 2x mode)
                    nc.vector.tensor_scalar(
                        out=oh[:], in0=e[:], scalar1=r[:], scalar2=None, op0=Alu.mult
                    )
                    if k > 0:
                        nc.gpsimd.tensor_add(khot[:], khot[:], oh[:])
                    if k < K - 1:
                        s = small_pool.tile([128, 1], f32, tag="s")
                        # e = (e * 1) * w2 (in place) ; s = rowsum(e)
                        nc.vector.scalar_tensor_tensor(
                            out=e[:], in0=e[:], scalar=1.0, in1=w2[:],
                            op0=Alu.mult, op1=Alu.mult, accum_out=s[:],
                        )
                return khot

            def bisect_one(t_idx, khot):
                lo = None
                for p in range(NPROBE):
                    c_p = 0.5 * (2.0 ** -(p + 1))
                    mid = small_pool.tile([128, 1], f32, tag="mid")
                    if lo is None:
                        nc.vector.memset(mid[:], c_p)
                    else:
                        nc.vector.tensor_scalar(
                            out=mid[:], in0=lo[:], scalar1=c_p, scalar2=None,
                            op0=Alu.add,
                        )
                    sgn = sgn_pool.tile([128, N], f32, tag="sgn")
                    cnt = small_pool.tile([128, 1], f32, tag="cnt")
                    # S' = sum(sign(mid - khot)); count(>mid)>=20 <=> S'<=N-2K
                    nc.scalar.activation(
                        sgn[:], khot[:], Act.Sign, bias=mid[:],
                        scale=-1.0, accum_out=cnt[:],
                    )
                    ge = small_pool.tile([128, 1], f32, tag="ge")
                    nc.vector.tensor_scalar(
                        out=ge[:], in0=cnt[:], scalar1=float(N - 2 * K),
                        scalar2=None, op0=Alu.is_le,
                    )
                    lo2 = small_pool.tile([128, 1], f32, tag="lo")
                    if lo is None:
                        nc.vector.tensor_scalar(
                            out=lo2[:], in0=ge[:], scalar1=c_p, scalar2=None,
                            op0=Alu.mult,
                        )
                    else:
                        nc.vector.scalar_tensor_tensor(
                            out=lo2[:], in0=ge[:], scalar=c_p, in1=lo[:],
                            op0=Alu.mult, op1=Alu.add,
                        )
                    lo = lo2
                v = oh_pool.tile([128, N], f32, tag="oh")
                # v = (khot + 1) - khot
                nc.vector.scalar_tensor_tensor(
                    out=v[:], in0=khot[:], scalar=1.0, in1=khot[:],
                    op0=Alu.add, op1=Alu.subtract,
                )
                o = o_pool.tile([128, N], f32, tag="o")
                # o = (khot > lo) * v
                nc.vector.scalar_tensor_tensor(
                    out=o[:], in0=khot[:], scalar=lo[:], in1=v[:],
                    op0=Alu.is_gt, op1=Alu.mult,
                )
                nc.sync.dma_start(out_s[t_idx], o[:])

            # stagger: emit tile t+1's main loop before tile t's bisection so
            # the static per-engine order interleaves each bisect chain with
            # the next tile's compute
            pending = None
            for t_idx in range(nt):
                khot = main_loop(t_idx)
                if pending is not None:
                    bisect_one(*pending)
                pending = (t_idx, khot)
            bisect_one(*pending)

    nc.compile()
    return nc


def _get_program():
    if "nc" not in _cached:
        _cached["nc"] = _build_program()
    return _cached["nc"]


def kernel(logits, gumbel, x=None, emb=None, _trace=False):
    from concourse.bass_utils import run_bass_kernel_spmd

    logits = np.ascontiguousarray(logits, dtype=np.float32)
    gumbel = np.ascontiguousarray(gumbel, dtype=np.float32)

    nc = _get_program()
    in_maps = []
    for c in range(NCORES):
        r0 = c * RPC
        lg = logits[r0:r0 + RPC].reshape(NBLK, 128, N)
        # tile t = blk*B + b  ->  gumbel[b, r0+blk*128 : r0+(blk+1)*128, :]
        gm = np.ascontiguousarray(
            gumbel[:, r0:r0 + RPC, :]            # [B, 256, N]
            .reshape(B, NBLK, 128, N)
            .transpose(1, 0, 2, 3)               # [NBLK, B, 128, N]
            .reshape(NT, 128, N)
        )
        in_maps.append({"logits_s": np.ascontiguousarray(lg), "gumbel_s": gm})

    res = None
    last_err = None
    for attempt in range(3):
        try:
            res = run_bass_kernel_spmd(nc, in_maps, list(range(NCORES)), trace=_trace)
            break
        except Exception as err:  # transient NRT/device failures: retry
            last_err = err
            import time as _time
            _time.sleep(5 * (attempt + 1))
    if res is None:
        raise last_err

    edge_weight = np.empty((B, N, N), dtype=np.float32)
    for c in range(NCORES):
        r0 = c * RPC
        o = res.results[c]["out_s"].reshape(NBLK, B, 128, N)
        edge_weight[:, r0:r0 + RPC, :] = o.transpose(1, 0, 2, 3).reshape(B, RPC, N)

    idx = np.arange(N, dtype=np.int32)
    edge_index = np.stack([np.tile(idx, N), np.repeat(idx, N)])
    out = (edge_index, edge_weight.reshape(B, N * N))
    if _trace:
        return out, res
    return out
